# revision 3
# baseline (speedup 1.0000x reference)
"""Sparse Adagrad (Habana-style) on 8 Trainium2 NeuronCores.

Strategy: row-shard the embedding tables (weights/moments) across the 8
cores by index range (62500 rows each, padded to 63488 = 128*496). The
host routes each valid gradient row to its owning core. On device, each
core sweeps its table shard once with large contiguous DMAs; the sparse
scatter-add (with duplicate indices) is done with one-hot matmuls on the
TensorEngine accumulating into PSUM, so duplicates sum natively.

Table layout per core: row r -> SBUF partition p = r // 496, free offset
j = r % 496 (so a [63488, 64] f32 shard is exactly a [128, 496*64] SBUF
sweep with contiguous per-partition DMA).

Per block j (the 128 rows {p*496 + j}), the host packs the gradient rows
whose local index maps to block j into up to CPB chunks of 128 "slots"
(slot -> partition). A one-hot matrix A[slot, p] = (strip(slot) == p)
is built on device via is_equal against an iota, and
    psum_m[p, :] += A.T @ g2_chunk      (moment increments, Sum g^2)
    psum_g[p, :] += A.T @ g_chunk       (gradient sums, Sum g)
Then the update (denominator uses the fully accumulated moment, and it
is constant across duplicates so it factors out of the sum):
    m' = m + psum_m
    w' = w - lr * psum_g / sqrt(m' + 1e-20)
"""

import sys

for _p in ("/opt/trn_rl_repo", "/root/.axon_site/_ro/trn_rl_repo"):
    if _p not in sys.path:
        sys.path.insert(0, _p)

import numpy as np

P = 128          # SBUF partitions / matmul contraction
D = 64           # embedding dim
NCORES = 8
VC = 62500       # table rows per core
R = 496          # rows per strip (= blocks per core); 128*496 = 63488 >= VC
PADV = P * R     # padded rows per core
JSUB = 16        # blocks per sweep iteration (PSUM limited)
NIT = R // JSUB  # 31 sweep iterations

_program_cache = {}


def _build_program(cpb):
    from concourse import bacc, mybir
    import concourse.tile as tile

    f32 = mybir.dt.float32
    nc = bacc.Bacc("TRN2", target_bir_lowering=False, debug=False,
                   num_devices=NCORES)

    w_in = nc.dram_tensor("w_in", [P, R * D], f32, kind="ExternalInput")
    m_in = nc.dram_tensor("m_in", [P, R * D], f32, kind="ExternalInput")
    g_in = nc.dram_tensor("g_in", [P, R * cpb * D], f32, kind="ExternalInput")
    midx = nc.dram_tensor("midx", [P, R * cpb], f32, kind="ExternalInput")
    lr_in = nc.dram_tensor("lr", [1, 1], f32, kind="ExternalInput")
    w_out = nc.dram_tensor("w_out", [P, R * D], f32, kind="ExternalOutput")
    m_out = nc.dram_tensor("m_out", [P, R * D], f32, kind="ExternalOutput")

    with tile.TileContext(nc) as tc:
        with tc.tile_pool(name="consts", bufs=1) as consts, \
             tc.tile_pool(name="sbuf", bufs=2) as pool, \
             tc.tile_pool(name="psum", bufs=2, space="PSUM") as psum:
            iota_i = consts.tile([P, P], mybir.dt.int32)
            nc.gpsimd.iota(iota_i[:], pattern=[[1, P]], base=0,
                           channel_multiplier=0)
            iota_f = consts.tile([P, P], f32)
            nc.vector.tensor_copy(iota_f[:], iota_i[:])

            eps_t = consts.tile([P, 1], f32)
            nc.gpsimd.memset(eps_t[:], 1e-20)

            neg_lr = consts.tile([P, 1], f32)
            lr_t = consts.tile([P, 1], f32)
            nc.sync.dma_start(out=lr_t[:], in_=lr_in[:].to_broadcast((P, 1)))
            nc.vector.tensor_scalar_mul(neg_lr[:], lr_t[:], -1.0)

            midx_s = consts.tile([P, R * cpb], f32)
            nc.sync.dma_start(out=midx_s[:], in_=midx[:])

            for it in range(NIT):
                c0, c1 = it * JSUB * D, (it + 1) * JSUB * D
                s0, s1 = it * JSUB * cpb * D, (it + 1) * JSUB * cpb * D
                k0 = it * JSUB * cpb

                w_s = pool.tile([P, JSUB * D], f32)
                nc.sync.dma_start(out=w_s[:], in_=w_in[:, c0:c1])
                m_s = pool.tile([P, JSUB * D], f32)
                nc.sync.dma_start(out=m_s[:], in_=m_in[:, c0:c1])
                g_s = pool.tile([P, JSUB * cpb * D], f32)
                nc.sync.dma_start(out=g_s[:], in_=g_in[:, s0:s1])

                g2_s = pool.tile([P, JSUB * cpb * D], f32)
                nc.scalar.square(g2_s[:], g_s[:])

                a_s = pool.tile([P, JSUB * cpb, P], f32)
                nc.vector.tensor_tensor(
                    out=a_s[:],
                    in0=midx_s[:, k0:k0 + JSUB * cpb, None].broadcast_to(
                        (P, JSUB * cpb, P)),
                    in1=iota_f[:, None, :].broadcast_to((P, JSUB * cpb, P)),
                    op=mybir.AluOpType.is_equal,
                )

                psum_m = psum.tile([P, JSUB * D], f32)
                psum_g = psum.tile([P, JSUB * D], f32)
                for jj in range(JSUB):
                    for c in range(cpb):
                        k = jj * cpb + c
                        nc.tensor.matmul(
                            out=psum_m[:, jj * D:(jj + 1) * D],
                            lhsT=a_s[:, k, :],
                            rhs=g2_s[:, k * D:(k + 1) * D],
                            start=(c == 0), stop=(c == cpb - 1),
                        )
                    for c in range(cpb):
                        k = jj * cpb + c
                        nc.tensor.matmul(
                            out=psum_g[:, jj * D:(jj + 1) * D],
                            lhsT=a_s[:, k, :],
                            rhs=g_s[:, k * D:(k + 1) * D],
                            start=(c == 0), stop=(c == cpb - 1),
                        )

                m_n = pool.tile([P, JSUB * D], f32)
                nc.vector.tensor_add(m_n[:], m_s[:], psum_m[:])
                nc.sync.dma_start(out=m_out[:, c0:c1], in_=m_n[:])

                s_t = pool.tile([P, JSUB * D], f32)
                nc.scalar.activation(s_t[:], m_n[:],
                                     mybir.ActivationFunctionType.Sqrt,
                                     bias=eps_t[:])
                r_t = pool.tile([P, JSUB * D], f32)
                nc.vector.reciprocal_approx_fast(out=r_t[:], in_=s_t[:])
                t_t = pool.tile([P, JSUB * D], f32)
                nc.vector.tensor_mul(t_t[:], r_t[:], psum_g[:])
                w_n = pool.tile([P, JSUB * D], f32)
                nc.vector.affine_then_add(out=w_n[:], in0=t_t[:], in1=w_s[:],
                                          scale=neg_lr[:], bias=0.0)
                nc.sync.dma_start(out=w_out[:, c0:c1], in_=w_n[:])

    nc.compile()
    return nc


def get_program(cpb):
    if cpb not in _program_cache:
        _program_cache[cpb] = _build_program(cpb)
    return _program_cache[cpb]


def prepare_inputs(gradients, weights, moments, indices, learning_rate,
                   valid_count):
    """Host-side routing: shard tables by row range, route gradient rows to
    owning cores, pack into the block/slot layout the device sweep expects."""
    g = np.ascontiguousarray(np.asarray(gradients, dtype=np.float32))
    w = np.asarray(weights, dtype=np.float32)
    m = np.asarray(moments, dtype=np.float32)
    idx = np.asarray(indices).astype(np.int64)
    vc = int(valid_count)
    lr = np.float32(np.asarray(learning_rate).reshape(-1)[0])

    idxv = idx[:vc]
    owner = idxv // VC
    loc = idxv - owner * VC
    j = loc % R
    mstrip = loc // R

    group = owner * R + j
    counts = np.bincount(group, minlength=NCORES * R)
    order = np.argsort(group, kind="stable")
    starts = np.concatenate(([0], np.cumsum(counts)[:-1]))
    rank = np.empty(vc, dtype=np.int64)
    rank[order] = np.arange(vc, dtype=np.int64) - starts[group[order]]

    cpb = max(1, -(-int(counts.max()) // P))  # ceil(maxcnt / 128)

    colidx = j * cpb + rank // P
    part = rank % P

    g_dev = np.zeros((NCORES, P, R * cpb, D), dtype=np.float32)
    g_dev[owner, part, colidx] = g[:vc]
    g_dev = g_dev.reshape(NCORES, P, R * cpb * D)

    midx_dev = np.zeros((NCORES, P, R * cpb), dtype=np.float32)
    midx_dev[owner, part, colidx] = mstrip.astype(np.float32)

    w_dev = np.zeros((NCORES, PADV, D), dtype=np.float32)
    w_dev[:, :VC] = w.reshape(NCORES, VC, D)
    w_dev = w_dev.reshape(NCORES, P, R * D)
    m_dev = np.zeros((NCORES, PADV, D), dtype=np.float32)
    m_dev[:, :VC] = m.reshape(NCORES, VC, D)
    m_dev = m_dev.reshape(NCORES, P, R * D)

    lr_arr = np.full((1, 1), lr, dtype=np.float32)

    in_maps = [
        {
            "w_in": w_dev[c],
            "m_in": m_dev[c],
            "g_in": g_dev[c],
            "midx": midx_dev[c],
            "lr": lr_arr,
        }
        for c in range(NCORES)
    ]
    return in_maps, cpb


def assemble_outputs(results):
    w_new = np.empty((NCORES * VC, D), dtype=np.float32)
    m_new = np.empty((NCORES * VC, D), dtype=np.float32)
    for c in range(NCORES):
        w_new[c * VC:(c + 1) * VC] = \
            results[c]["w_out"].reshape(PADV, D)[:VC]
        m_new[c * VC:(c + 1) * VC] = \
            results[c]["m_out"].reshape(PADV, D)[:VC]
    return w_new, m_new


def kernel(gradients, weights, moments, indices, learning_rate, valid_count):
    from concourse.bass_utils import run_bass_kernel_spmd

    in_maps, cpb = prepare_inputs(gradients, weights, moments, indices,
                                  learning_rate, valid_count)
    nc = get_program(cpb)
    res = run_bass_kernel_spmd(nc, in_maps, core_ids=list(range(NCORES)))
    return assemble_outputs(res.results)
